# revision 11
# baseline (speedup 1.0000x reference)
"""Self-contained 8-core Trainium2 Bass kernel for nn_GATNet (GAT x2 + 2-layer
transformer (seq_len=1) + global max pool).

Strategy:
- GAT1 channel-sharded: each core computes xs1 = x1 @ W1[:, cols_d] for ALL
  nodes (640 of 5120 channels), writes rows to DRAM, dma_gathers per-edge rows
  (sorted by dst, tiled 128 dst-nodes / 6 chunks of 128 edge-slots), computes
  attention via one-hot scatter-matrix matmuls, aggregates on the PE.
- xs2 = h1 @ W2 partial per core -> ReduceScatter(+AllGather) in bf16.
- GAT2 + transformer + pooling node-sharded (512 nodes/core).
- All matmuls bf16 x bf16 -> f32 PSUM; softmax/LN arithmetic f32.
Host side does only: int index preprocessing (edge sort by dst), weight-only
constant folding, dtype casts/layout, and output reassembly.
"""
import sys
import types
import numpy as np
import ml_dtypes
from contextlib import ExitStack

import concourse.bass as bass
import concourse.tile as tile
from concourse import bacc, mybir
from concourse import bass_utils

try:  # optional NTFF profiling plumbing (no-op if unavailable)
    import antenv.axon_hooks  # noqa: F401
except ImportError:
    _h = types.ModuleType("antenv.axon_hooks")
    _h._hook = None
    _h.set_axon_ntff_profile_hook = lambda hk: setattr(_h, "_hook", hk)
    _h.get_axon_ntff_profile_hook = lambda: _h._hook
    sys.modules["antenv.axon_hooks"] = _h
    try:
        from trn_agent_boot.trn_boot import _ntff_profile_via_ctypes
        _h.set_axon_ntff_profile_hook(
            _ntff_profile_via_ctypes("/opt/axon/libaxon_pjrt.so"))
        bass_utils.upload_artifacts = lambda tmpdir: f"local://{tmpdir}"
    except Exception:
        pass

bf16 = mybir.dt.bfloat16
f32 = mybir.dt.float32
i16 = mybir.dt.int16

N, E, B = 4096, 16384, 64
H1, C, D = 10, 512, 512
NDEV = 8
CSH = 640            # GAT1 channels per device
CPH = CSH // H1      # 64 channels per head per device
NT = N // 128        # 32 node tiles
NCH = 6              # chunks (of 128 edge slots) per node tile, uniform
T1C = NT * NCH       # 192 total GAT1 chunks
LT = 4               # local node tiles per device
C2C = LT * NCH       # GAT2 chunks per device
AluOp = mybir.AluOpType
Act = mybir.ActivationFunctionType

_cache = {}


def _dev_cols(d):
    return np.concatenate(
        [h * C + np.arange(d * CPH, (d + 1) * CPH) for h in range(H1)])


def _wrap_idx(idx):
    """[n] int -> [128, n//16] int16 gather-index layout."""
    n = len(idx)
    w = np.zeros((128, n // 16), np.int16)
    base = idx.reshape(-1, 16).T.astype(np.int16)   # [16, n//16]
    for r in range(8):
        w[r * 16:(r + 1) * 16, :] = base
    return w


def _host_prep(inputs):
    ei = np.asarray(inputs["edge_index"])
    ea = np.asarray(inputs["edge_attr"]).astype(np.float32)
    src0, dst0 = ei[0].astype(np.int64), ei[1].astype(np.int64)

    src_all = np.concatenate([src0, np.arange(N)])
    dst_all = np.concatenate([dst0, np.arange(N)])
    isreal = np.concatenate([np.ones(E, np.float32), np.zeros(N, np.float32)])
    eid = np.arange(E + N)
    order = np.argsort(dst_all, kind="stable")
    src_s, dst_s, isreal_s, eid_s = (src_all[order], dst_all[order],
                                     isreal[order], eid[order])
    seg = np.searchsorted(dst_s, np.arange(0, N + 1, 128))

    NS = NT * NCH * 128
    slot_src = np.zeros(NS, np.int64)
    slot_dstloc = np.full(NS, 255.0, np.float32)
    slot_isreal = np.zeros(NS, np.float32)
    slot_isloop = np.zeros(NS, np.float32)
    slot_eid = np.full(NS, -1, np.int64)
    slot_ea = np.zeros((NS, 11), np.float32)
    ea_all = np.concatenate([ea, np.zeros((N, 11), np.float32)])
    for t in range(NT):
        lo, hi = seg[t], seg[t + 1]
        n = hi - lo
        assert n <= NCH * 128, f"tile {t} has {n} slots > {NCH*128}"
        s = t * NCH * 128
        slot_src[s:s + n] = src_s[lo:hi]
        slot_dstloc[s:s + n] = dst_s[lo:hi] - t * 128
        slot_isreal[s:s + n] = isreal_s[lo:hi]
        slot_isloop[s:s + n] = (isreal_s[lo:hi] == 0).astype(np.float32)
        slot_eid[s:s + n] = eid_s[lo:hi]
        slot_ea[s:s + n] = ea_all[eid_s[lo:hi]]

    # weight folds (f32 host math on weights only)
    W1 = np.asarray(inputs["W1"], np.float32)
    Ms = np.einsum("fhc,hc->fh", W1.reshape(93, H1, C),
                   np.asarray(inputs["as1"], np.float32))
    Md = np.einsum("fhc,hc->fh", W1.reshape(93, H1, C),
                   np.asarray(inputs["ad1"], np.float32))
    We1 = np.asarray(inputs["We1"], np.float32)
    Me1 = np.einsum("fhc,hc->fh", We1.reshape(11, H1, C),
                    np.asarray(inputs["ae1"], np.float32))
    W2 = np.asarray(inputs["W2"], np.float32)
    me2 = np.asarray(inputs["We2"], np.float32) @ np.asarray(
        inputs["ae2"], np.float32)[0]
    w2s = W2 @ np.asarray(inputs["as2"], np.float32)[0]
    w2d = W2 @ np.asarray(inputs["ad2"], np.float32)[0]

    # per-slot folded edge logit terms (fold of weights with edge attrs)
    se1 = slot_ea @ Me1                     # [NS, 10]
    se2 = slot_ea @ me2                     # [NS]

    def slotgrid(v, w=None):
        """[NS(,w)] -> [128, NT, NCH(,w)] partition layout."""
        if w is None:
            return np.ascontiguousarray(
                v.reshape(NT, NCH, 128).transpose(2, 0, 1))
        return np.ascontiguousarray(
            v.reshape(NT, NCH, 128, w).transpose(2, 0, 1, 3))

    b16 = ml_dtypes.bfloat16
    common = {
        "x1T": np.ascontiguousarray(
            np.asarray(inputs["x1"], np.float32).T).astype(b16),
        "easlot": np.concatenate(
            [slot_isreal[:, None], se1, se2[:, None],
             np.zeros((NS, 4), np.float32)], axis=1
        ).astype(b16).reshape(NT, NCH, 128, 16).transpose(2, 0, 1, 3).copy(),
        "se1f": slotgrid(se1, 10).astype(np.float32),
        "dstloc": slotgrid(slot_dstloc),
        "isloop": slotgrid(slot_isloop),
        "isloop10": np.repeat(slotgrid(slot_isloop)[:, :, :, None], 10, axis=3
                              ).astype(np.float32).copy(),
        "iota128": np.tile(np.arange(128, dtype=np.float32)[None, :], (128, 1)),
        "ident": np.eye(128, dtype=b16),
        "ones_k": np.ones((1, 128), b16),
        "onescol": np.ones((128, 1), b16),
        "idx1": _wrap_idx(slot_src),
        "b2col": np.asarray(inputs["b2"], np.float32).reshape(4, 128).T.copy(),
    }
    # transformer weights
    iw = np.asarray(inputs["in_w"], np.float32)
    Wv = iw[:, :, 2 * D:]                               # [2, 512, 512]
    common["Wv"] = np.ascontiguousarray(
        Wv.reshape(2, 4, 128, 512).transpose(2, 0, 1, 3)).astype(b16)
    Wo = np.asarray(inputs["out_w"], np.float32)
    common["Wo"] = np.ascontiguousarray(
        Wo.reshape(2, 4, 128, 512).transpose(2, 0, 1, 3)).astype(b16)
    W1f = np.asarray(inputs["f1w"], np.float32)
    common["W1f"] = np.ascontiguousarray(
        W1f.reshape(2, 4, 128, 2048).transpose(2, 0, 1, 3)).astype(b16)
    W2f = np.asarray(inputs["f2w"], np.float32)
    common["W2f"] = np.ascontiguousarray(
        W2f.reshape(2, 16, 128, 512).transpose(2, 0, 1, 3)).astype(b16)

    def col_layout(v, nc_):
        # [2, nc_*128] -> [128, 2, nc_]
        return np.ascontiguousarray(
            v.reshape(2, nc_, 128).transpose(2, 0, 1)).astype(np.float32)

    common["bvcol"] = col_layout(np.asarray(inputs["in_b"], np.float32)[:, 2 * D:], 4)
    common["bocol"] = col_layout(np.asarray(inputs["out_b"], np.float32), 4)
    common["f1bcol"] = col_layout(np.asarray(inputs["f1b"], np.float32), 16)
    common["f2bcol"] = col_layout(np.asarray(inputs["f2b"], np.float32), 4)
    common["g1col"] = col_layout(np.asarray(inputs["ln1g"], np.float32), 4)
    common["h1col"] = col_layout(np.asarray(inputs["ln1b"], np.float32), 4)
    common["g2col"] = col_layout(np.asarray(inputs["ln2g"], np.float32), 4)
    common["h2col"] = col_layout(np.asarray(inputs["ln2b"], np.float32), 4)

    in_maps = []
    b1 = np.asarray(inputs["b1"], np.float32)
    for d in range(NDEV):
        cols = _dev_cols(d)
        m = dict(common)
        m["W1aug"] = np.concatenate(
            [W1[:, cols], Ms, Md], axis=1).astype(b16)          # [93, 660]
        m["W2aug"] = np.ascontiguousarray(np.concatenate(
            [W2[cols], w2s[cols, None], w2d[cols, None],
             np.zeros((CSH, 2), np.float32)], axis=1
        ).reshape(5, 128, 516).transpose(1, 0, 2)).astype(b16)  # [128,5,516]
        m["b1col"] = np.ascontiguousarray(
            b1[cols].reshape(5, 128).T).astype(np.float32)      # [128, 5]
        # GAT2 per-core slot structure: local tiles = global tiles 4d..4d+3
        gsl = slice(4 * d * NCH * 128, (4 * d + 4) * NCH * 128)
        m["idx2"] = _wrap_idx(slot_src[gsl])
        m["dstloc2"] = slotgrid(slot_dstloc)[:, 4 * d:4 * d + 4, :].copy()
        m["isloop2"] = slotgrid(slot_isloop)[:, 4 * d:4 * d + 4, :].copy()
        m["ease2f"] = slotgrid(se2)[:, 4 * d:4 * d + 4, :].astype(np.float32).copy()
        in_maps.append(m)

    meta = dict(slot_eid=slot_eid)
    return in_maps, meta


DEBUG = False


def _build_nc():
    nc = bacc.Bacc("TRN2", target_bir_lowering=False, debug=True)

    def inp(name, shape, dt):
        return nc.dram_tensor(name, list(shape), dt, kind="ExternalInput")

    x1T = inp("x1T", (93, N), bf16)
    W1aug = inp("W1aug", (93, 660), bf16)
    easlot = inp("easlot", (128, NT, NCH, 16), bf16)
    se1f = inp("se1f", (128, NT, NCH, 10), f32)
    dstloc = inp("dstloc", (128, NT, NCH), f32)
    isloop = inp("isloop", (128, NT, NCH), f32)
    isloop10 = inp("isloop10", (128, NT, NCH, 10), f32)
    iota128 = inp("iota128", (128, 128), f32)
    ident = inp("ident", (128, 128), bf16)
    ones_k = inp("ones_k", (1, 128), bf16)
    onescol = inp("onescol", (128, 1), bf16)
    idx1 = inp("idx1", (128, T1C * 8), i16)
    idx2 = inp("idx2", (128, C2C * 8), i16)
    dstloc2 = inp("dstloc2", (128, LT, NCH), f32)
    isloop2 = inp("isloop2", (128, LT, NCH), f32)
    ease2f = inp("ease2f", (128, LT, NCH), f32)
    W2aug = inp("W2aug", (128, 5, 516), bf16)
    b1col = inp("b1col", (128, 5), f32)
    b2col = inp("b2col", (128, 4), f32)
    Wv = inp("Wv", (128, 2, 4, 512), bf16)
    Wo = inp("Wo", (128, 2, 4, 512), bf16)
    W1f = inp("W1f", (128, 2, 4, 2048), bf16)
    W2f = inp("W2f", (128, 2, 16, 512), bf16)
    bvcol = inp("bvcol", (128, 2, 4), f32)
    bocol = inp("bocol", (128, 2, 4), f32)
    f1bcol = inp("f1bcol", (128, 2, 16), f32)
    f2bcol = inp("f2bcol", (128, 2, 4), f32)
    g1col = inp("g1col", (128, 2, 4), f32)
    h1col = inp("h1col", (128, 2, 4), f32)
    g2col = inp("g2col", (128, 2, 4), f32)
    h2col = inp("h2col", (128, 2, 4), f32)

    pooled = nc.dram_tensor("pooled", [128, 4, 8], f32, kind="ExternalOutput")
    alpha2o = nc.dram_tensor("alpha2o", [128, C2C], f32, kind="ExternalOutput")
    if DEBUG:
        dbg_xs1 = nc.dram_tensor("dbg_xs1", [N, 768], bf16, kind="ExternalOutput")
        dbg_cc = nc.dram_tensor("dbg_cc", [N, 640], bf16, kind="ExternalOutput")
        dbg_ag = nc.dram_tensor("dbg_ag", [N, 640], bf16, kind="ExternalOutput")
        dbg_t0 = nc.dram_tensor("dbg_t0", [128, 64], f32, kind="ExternalOutput")
        dbg_g = nc.dram_tensor("dbg_g", [128, NCH, 768], bf16, kind="ExternalOutput")
        dbg_h1 = nc.dram_tensor("dbg_h1", [128, 5, N], bf16, kind="ExternalOutput")
        dbg_h2 = nc.dram_tensor("dbg_h2", [128, 4, 512], bf16, kind="ExternalOutput")
        dbg_ph = nc.dram_tensor("dbg_ph", [128, NCH, 10], f32, kind="ExternalOutput")

    xs1_dram = nc.dram_tensor("xs1_scratch", [N, 768], bf16)
    cc_in = nc.dram_tensor("cc_in", [N, 640], bf16)
    rs_out = nc.dram_tensor("rs_out", [N // NDEV, 640], bf16)
    ag_out = nc.dram_tensor("ag_out", [N, 640], bf16, addr_space="Shared")

    with tile.TileContext(nc) as tc, ExitStack() as top:
        cst = top.enter_context(tc.tile_pool(name="cst", bufs=1))
        mid_ctx = ExitStack()
        mid = mid_ctx.enter_context(tc.tile_pool(name="mid", bufs=1))

        def load(dram_t, shape, dt, pool=cst):
            t = pool.tile(list(shape), dt, name=dram_t.name + "_sb")
            nc.sync.dma_start(t[:], dram_t[:])
            return t

        x1T_s = load(x1T, (93, N), bf16, mid)
        W1aug_s = load(W1aug, (93, 660), bf16, mid)
        easlot_s = load(easlot, (128, NT, NCH, 16), bf16, mid)
        se1f_s = load(se1f, (128, NT, NCH, 10), f32, mid)
        dstloc_s = load(dstloc, (128, NT, NCH), f32, mid)
        isloop_s = load(isloop, (128, NT, NCH), f32, mid)
        isloop10_s = load(isloop10, (128, NT, NCH, 10), f32, mid)
        iota_s = load(iota128, (128, 128), f32)
        ident_s = load(ident, (128, 128), bf16)
        onesk_s = load(ones_k, (1, 128), bf16)
        onesc_s = load(onescol, (128, 1), bf16)
        idx1_s = load(idx1, (128, T1C * 8), i16, mid)
        W2aug_s = load(W2aug, (128, 5, 516), bf16, mid)
        b1col_s = load(b1col, (128, 5), f32, mid)

        # stats living through phases B..D only
        sdst_all = mid.tile([128, NT, 10], f32)     # s_dst1 per tile
        se2q_all = mid.tile([128, NT], f32)         # se2_loop/8 per tile
        h1T = mid.tile([128, 5, N], bf16)           # GAT1 output, T-layout

        # ---------------- Phase B: xs1 for all tiles ----------------
        with ExitStack() as sb_:
            psB = sb_.enter_context(tc.tile_pool(name="psB", bufs=2, space="PSUM"))
            sbB = sb_.enter_context(tc.tile_pool(name="sbB", bufs=3))
            for t in range(NT):
                ps = psB.tile([128, 660], f32, name=f"xs1ps{t}", tag="xs1ps")
                lhs = x1T_s[:, t * 128:(t + 1) * 128]
                nc.tensor.matmul(ps[:, 0:512], lhs, W1aug_s[:, 0:512],
                                 start=True, stop=True)
                nc.tensor.matmul(ps[:, 512:660], lhs, W1aug_s[:, 512:660],
                                 start=True, stop=True)
                xb = sbB.tile([128, 650], bf16, name=f"xs1b{t}", tag="xs1b")
                nc.scalar.copy(xb[:, 0:650], ps[:, 0:650])
                nc.vector.tensor_copy(sdst_all[:, t, :], ps[:, 650:660])
                nc.sync.dma_start(xs1_dram[t * 128:(t + 1) * 128, 0:650], xb[:])

        # ---------------- Phase A/C/D per tile ----------------
        with ExitStack() as sc_:
            sbS = sc_.enter_context(tc.tile_pool(name="sbS", bufs=2 * NCH + 2))
            sbG = sc_.enter_context(tc.tile_pool(name="sbG", bufs=3))
            sbW = sc_.enter_context(tc.tile_pool(name="sbW", bufs=4))
            sbU = sc_.enter_context(tc.tile_pool(name="sbU", bufs=2 * NCH + 2))
            psT = sc_.enter_context(tc.tile_pool(name="psT", bufs=2, space="PSUM"))
            psB2 = sc_.enter_context(tc.tile_pool(name="psB2", bufs=1, space="PSUM"))
            psS = sc_.enter_context(tc.tile_pool(name="psS", bufs=2, space="PSUM"))
            psA = sc_.enter_context(tc.tile_pool(name="psA", bufs=1, space="PSUM"))

            for t in range(NT):
                S_l, SkT_l = [], []
                esum = psS.tile([128, 12], f32, name=f"esum{t}", tag="stat")
                for j in range(NCH):
                    S_k = sbS.tile([128, 128], bf16, name=f"S{t}_{j}", tag="S")
                    nc.vector.tensor_scalar(
                        S_k[:], iota_s[:], dstloc_s[:, t, j:j + 1], None,
                        AluOp.is_equal)
                    tps = psT.tile([128, 128], bf16, name=f"tp{t}_{j}", tag="trbc")
                    nc.tensor.transpose(tps[:], S_k[:], ident_s[:])
                    SkT = sbS.tile([128, 128], bf16, name=f"ST{t}_{j}", tag="ST")
                    nc.scalar.copy(SkT[:], tps[:])
                    nc.tensor.matmul(esum[:], S_k[:], easlot_s[:, t, j, 0:12],
                                     start=(j == 0), stop=(j == NCH - 1))
                    S_l.append(S_k)
                    SkT_l.append(SkT)
                # loop stats
                recipc = sbW.tile([128, 1], f32, name=f"rc{t}", tag="rc")
                nc.vector.tensor_scalar_max(recipc[:], esum[:, 0:1], 1.0)
                nc.vector.reciprocal(recipc[:], recipc[:])
                sdaug = sbW.tile([128, 20], f32, name=f"sda{t}", tag="sda")
                nc.vector.tensor_copy(sdaug[:, 0:10], sdst_all[:, t, :])
                nc.vector.tensor_scalar_mul(sdaug[:, 10:20], esum[:, 1:11],
                                            recipc[:])
                nc.vector.tensor_scalar(se2q_all[:, t:t + 1], esum[:, 11:12],
                                        recipc[:], 0.125, AluOp.mult,
                                        AluOp.mult)
                sdaugb = sbW.tile([128, 20], bf16, name=f"sdab{t}", tag="sdab")
                nc.vector.tensor_copy(sdaugb[:], sdaug[:])

                # gather
                G = sbG.tile([128, NCH, 768], bf16, name=f"G{t}", tag="G")
                nc.gpsimd.dma_gather(
                    G[:], xs1_dram[:], idx1_s[:, t * NCH * 8:(t + 1) * NCH * 8],
                    NCH * 128, NCH * 128, 768)

                denom = psS.tile([128, 10], f32, name=f"den{t}", tag="stat")
                bca = psB2.tile([128, NCH, 20], f32, name=f"bca{t}", tag="bca")
                nc.vector.memset(bca[:], 0.0)
                for j in range(NCH):
                    nc.tensor.matmul(bca[:, j, :], SkT_l[j][:], sdaugb[:],
                                     start=False, stop=False,
                                     skip_group_check=True)
                u = sbU.tile([128, NCH, 10], f32, name=f"u{t}", tag="u")
                nc.vector.tensor_tensor(u[:], bca[:, :, 0:10],
                                        se1f_s[:, t, :, :], AluOp.add)
                u2 = sbU.tile([128, NCH, 10], f32, name=f"u2{t}", tag="u2")
                nc.vector.tensor_tensor(u2[:], bca[:, :, 10:20],
                                        isloop10_s[:, t, :, :], AluOp.mult)
                nc.vector.tensor_tensor(u[:], u[:], u2[:], AluOp.add)
                gs = sbU.tile([128, NCH, 10], f32, name=f"gs{t}", tag="gs")
                nc.scalar.copy(gs[:], G[:, :, 640:650])
                nc.vector.tensor_tensor(u[:], u[:], gs[:], AluOp.add)
                nc.vector.tensor_scalar_mul(u2[:], u[:], 0.2)
                nc.vector.tensor_tensor(u[:], u[:], u2[:], AluOp.max)
                ph = sbU.tile([128, NCH, 10], f32, name=f"ph{t}", tag="ph")
                nc.scalar.activation(ph[:], u[:], Act.Exp)
                phb = sbU.tile([128, NCH, 10], bf16, name=f"phb{t}", tag="phb")
                nc.vector.tensor_copy(phb[:], ph[:])
                for j in range(NCH):
                    nc.tensor.matmul(denom[:], S_l[j][:], phb[:, j, :],
                                     start=(j == 0), stop=(j == NCH - 1))
                if DEBUG and t == 0:
                    phat_l = [ph[:, _j, :] for _j in range(NCH)]
                recip = sbW.tile([128, 10], f32, name=f"rp{t}", tag="rp")
                nc.vector.reciprocal(recip[:], denom[:])
                recipb = sbW.tile([128, 10], bf16, name=f"rpb{t}", tag="rpb")
                nc.vector.tensor_copy(recipb[:], recip[:])
                if DEBUG and t == 0:
                    d0 = sbW.tile([128, 64], f32, name="dbg_t0sb", tag="d0")
                    nc.vector.tensor_copy(d0[:, 0:12], esum[:])
                    nc.vector.tensor_copy(d0[:, 12:32], sdaug[:])
                    nc.vector.tensor_copy(d0[:, 32:42], denom[:])
                    nc.vector.tensor_copy(d0[:, 42:52], recip[:])
                    nc.vector.tensor_copy(d0[:, 52:62], sdst_all[:, 0, :])
                    nc.vector.tensor_copy(d0[:, 62:63], se2q_all[:, 0:1])
                    nc.vector.memset(d0[:, 63:64], 0.0)
                    nc.sync.dma_start(dbg_t0[:], d0[:])
                    nc.sync.dma_start(dbg_g[:], G[:])
                    for _j in range(NCH):
                        nc.sync.dma_start(dbg_ph[:, _j, :], phat_l[_j][:])

                agg = psA.tile([128, 5, 128], f32, name=f"agg{t}", tag="agg")
                nc.vector.memset(agg[:], 0.0)
                rba = psB2.tile([128, NCH, 10], f32, name=f"rba{t}", tag="rba")
                nc.vector.memset(rba[:], 0.0)
                for j in range(NCH):
                    nc.tensor.matmul(rba[:, j, :], SkT_l[j][:], recipb[:],
                                     start=False, stop=False,
                                     skip_group_check=True)
                alb = sbU.tile([128, NCH, 10], bf16, name=f"alb{t}", tag="alb")
                nc.vector.tensor_tensor(alb[:], ph[:], rba[:], AluOp.mult)
                for j in range(NCH):
                    Gw = sbG.tile([128, 640], bf16, name=f"Gw{t}_{j}", tag="Gw")
                    nc.vector.tensor_tensor(
                        Gw[:].rearrange("p (h c) -> p h c", h=H1),
                        G[:, j, 0:640].rearrange("p (h c) -> p h c", h=H1),
                        alb[:, j, :, None].broadcast_to([128, H1, CPH]),
                        AluOp.mult)
                    for m in range(5):
                        nc.tensor.matmul(
                            agg[:, m, :], Gw[:, m * 128:(m + 1) * 128],
                            S_l[j][:], start=False, stop=False,
                            skip_group_check=True)
                # stage accumulator to SBUF so the PSUM bank frees early
                stg = sbW.tile([128, 5, 128], f32, name=f"stg{t}", tag="stg")
                nc.scalar.copy(stg[:], agg[:])
                # elu + bias -> h1T (batched over the 5 m-chunks)
                yb = sbW.tile([128, 5, 128], f32, name=f"yb{t}", tag="yb")
                nc.vector.tensor_tensor(
                    yb[:], stg[:],
                    b1col_s[:, :, None].broadcast_to([128, 5, 128]), AluOp.add)
                q = sbW.tile([128, 5, 128], f32, name=f"q{t}", tag="q")
                nc.vector.tensor_scalar_min(q[:], yb[:], 0.0)
                e = sbW.tile([128, 5, 128], f32, name=f"e{t}", tag="e")
                nc.scalar.activation(e[:], q[:], Act.Exp)
                nc.vector.tensor_scalar_max(yb[:], yb[:], 0.0)
                nc.vector.tensor_tensor(e[:], e[:], yb[:], AluOp.add)
                nc.vector.tensor_scalar_add(
                    h1T.rearrange("p m (nt x) -> p m nt x", x=128)[:, :, t, :],
                    e[:], -1.0)

                # Phase D: xs2 partial for this tile
                xs2p = psS.tile([128, 512], f32, name=f"xs2p{t}", tag="stat")
                xs2s = psS.tile([128, 4], f32, name=f"xs2s{t}", tag="stat")
                for kc in range(5):
                    lh = h1T[:, kc, t * 128:(t + 1) * 128]
                    nc.tensor.matmul(xs2p[:], lh, W2aug_s[:, kc, 0:512],
                                     start=(kc == 0), stop=(kc == 4))
                    nc.tensor.matmul(xs2s[:, 0:2], lh, W2aug_s[:, kc, 512:514],
                                     start=(kc == 0), stop=(kc == 4))
                ccs = sbG.tile([128, 640], bf16, name=f"cc{t}", tag="ccs")
                nc.scalar.copy(ccs[:, 0:512], xs2p[:])
                nc.scalar.copy(ccs[:, 512:514], xs2s[:, 0:2])
                nc.vector.tensor_copy(ccs[:, 514:515], se2q_all[:, t:t + 1])
                nc.vector.memset(ccs[:, 515:640], 0.0)
                nc.sync.dma_start(cc_in[t * 128:(t + 1) * 128, :], ccs[:])

        if DEBUG:
            nc.sync.dma_start(dbg_h1[:], h1T[:])
            nc.sync.dma_start(dbg_xs1[:], xs1_dram[:])
            nc.sync.dma_start(dbg_cc[:], cc_in[:])
        mid_ctx.close()

        # ---------------- collectives ----------------
        nc.gpsimd.collective_compute(
            "ReduceScatter", AluOp.add, replica_groups=[list(range(NDEV))],
            ins=[cc_in[:]], outs=[rs_out[:]])
        nc.gpsimd.collective_compute(
            "AllGather", AluOp.bypass, replica_groups=[list(range(NDEV))],
            ins=[rs_out[:]], outs=[ag_out[:]])

        # ---------------- Phase E: GAT2 ----------------
        with ExitStack() as se_:
            sbS2 = se_.enter_context(tc.tile_pool(name="sbS2", bufs=2 * NCH + 2))
            sbG2 = se_.enter_context(tc.tile_pool(name="sbG2", bufs=2))
            sbU2 = se_.enter_context(tc.tile_pool(name="sbU2", bufs=2 * NCH + 2))
            psT2 = se_.enter_context(tc.tile_pool(name="psT2", bufs=2, space="PSUM"))
            psS2 = se_.enter_context(tc.tile_pool(name="psS2", bufs=2, space="PSUM"))
            psA2 = se_.enter_context(tc.tile_pool(name="psA2", bufs=4, space="PSUM"))
            idx2_s = sbS2.tile([128, C2C * 8], i16, name="idx2_sb", tag="idx2")
            nc.sync.dma_start(idx2_s[:], idx2[:])
            dstloc2_s = sbS2.tile([128, LT, NCH], f32, name="dl2", tag="dl2")
            nc.sync.dma_start(dstloc2_s[:], dstloc2[:])
            isloop2_s = sbS2.tile([128, LT, NCH], f32, name="il2", tag="il2")
            nc.sync.dma_start(isloop2_s[:], isloop2[:])
            ease2_s = sbS2.tile([128, LT, NCH], f32, name="ea2", tag="ea2")
            nc.sync.dma_start(ease2_s[:], ease2f[:])
            b2c_s = sbS2.tile([128, 4], f32, name="b2c_s", tag="b2c")
            nc.sync.dma_start(b2c_s[:], b2col[:])
            sd2 = sbS2.tile([128, LT, 2], bf16, name="sd2", tag="sd2")
            for tl in range(LT):
                nc.sync.dma_start(sd2[:, tl, :],
                                  rs_out[tl * 128:(tl + 1) * 128, 513:515])
            h2T = cst.tile([128, 4, 512], bf16)
            alpha2sb = cst.tile([128, C2C], f32)
            for tl in range(LT):
                S_l, SkT_l = [], []
                for j in range(NCH):
                    S_k = sbS2.tile([128, 128], bf16, name=f"S2{tl}_{j}", tag="S2")
                    nc.vector.tensor_scalar(
                        S_k[:], iota_s[:], dstloc2_s[:, tl, j:j + 1], None,
                        AluOp.is_equal)
                    tps = psT2.tile([128, 128], bf16, name=f"tp2{tl}_{j}", tag="trbc2")
                    nc.tensor.transpose(tps[:], S_k[:], ident_s[:])
                    SkT = sbS2.tile([128, 128], bf16, name=f"ST2{tl}_{j}", tag="ST2")
                    nc.scalar.copy(SkT[:], tps[:])
                    S_l.append(S_k)
                    SkT_l.append(SkT)
                G2 = sbG2.tile([128, NCH, 640], bf16, name=f"G2{tl}", tag="G2")
                nc.gpsimd.dma_gather(
                    G2[:], ag_out[:], idx2_s[:, tl * NCH * 8:(tl + 1) * NCH * 8],
                    NCH * 128, NCH * 128, 640)
                den2 = psS2.tile([128, 1], f32, name=f"den2{tl}", tag="stat2")
                p2_l = []
                for j in range(NCH):
                    bc = psT2.tile([128, 2], f32, name=f"bc2{tl}_{j}", tag="trbc2")
                    nc.tensor.matmul(bc[:], SkT_l[j][:], sd2[:, tl, :],
                                     start=True, stop=True)
                    u = sbU2.tile([128, 1], f32, name=f"v{tl}_{j}", tag="v")
                    nc.scalar.copy(u[:], G2[:, j, 512:513])
                    nc.vector.tensor_tensor(u[:], u[:], bc[:, 0:1], AluOp.add)
                    nc.vector.tensor_tensor(u[:], u[:], ease2_s[:, tl, j:j + 1],
                                            AluOp.add)
                    u2 = sbU2.tile([128, 1], f32, name=f"v2{tl}_{j}", tag="v2")
                    nc.vector.tensor_scalar_mul(u2[:], bc[:, 1:2],
                                                isloop2_s[:, tl, j:j + 1])
                    nc.vector.tensor_tensor(u[:], u[:], u2[:], AluOp.add)
                    nc.vector.tensor_scalar_mul(u2[:], u[:], 0.2)
                    nc.vector.tensor_tensor(u[:], u[:], u2[:], AluOp.max)
                    p2 = sbU2.tile([128, 1], f32, name=f"p2{tl}_{j}", tag="p2")
                    nc.scalar.activation(p2[:], u[:], Act.Exp)
                    p2b = sbU2.tile([128, 1], bf16, name=f"p2b{tl}_{j}", tag="p2b")
                    nc.vector.tensor_copy(p2b[:], p2[:])
                    nc.tensor.matmul(den2[:], S_l[j][:], p2b[:],
                                     start=(j == 0), stop=(j == NCH - 1))
                    p2_l.append(p2)
                rc2 = sbU2.tile([128, 1], f32, name=f"rc2{tl}", tag="rc2")
                nc.vector.reciprocal(rc2[:], den2[:])
                rc2b = sbU2.tile([128, 1], bf16, name=f"rc2b{tl}", tag="rc2b")
                nc.vector.tensor_copy(rc2b[:], rc2[:])
                agg2 = [psA2.tile([128, 128], f32, name=f"agg2{tl}_{m}", tag="agg2")
                        for m in range(4)]
                for j in range(NCH):
                    rb = psT2.tile([128, 1], f32, name=f"rb2{tl}_{j}", tag="trbc2")
                    nc.tensor.matmul(rb[:], SkT_l[j][:], rc2b[:],
                                     start=True, stop=True)
                    kk = tl * NCH + j
                    nc.vector.tensor_tensor(alpha2sb[:, kk:kk + 1],
                                            p2_l[j][:], rb[:], AluOp.mult)
                    Gw = sbG2.tile([128, 512], bf16, name=f"Gw2{tl}_{j}", tag="Gw2")
                    nc.vector.tensor_scalar_mul(Gw[:], G2[:, j, 0:512],
                                                alpha2sb[:, kk:kk + 1])
                    for m in range(4):
                        nc.tensor.matmul(
                            agg2[m][:], Gw[:, m * 128:(m + 1) * 128],
                            S_l[j][:], start=(j == 0), stop=(j == NCH - 1))
                for m in range(4):
                    st2 = sbU2.tile([128, 128], f32, name=f"st2{tl}_{m}", tag="st2")
                    nc.scalar.copy(st2[:], agg2[m][:])
                    yb = sbU2.tile([128, 128], f32, name=f"yb2{tl}_{m}", tag="q2")
                    nc.vector.tensor_scalar_add(yb[:], st2[:],
                                                b2c_s[:, m:m + 1])
                    q = sbU2.tile([128, 128], f32, name=f"q2{tl}_{m}", tag="e2")
                    nc.vector.tensor_scalar_min(q[:], yb[:], 0.0)
                    e = sbU2.tile([128, 128], f32, name=f"e2b{tl}_{m}", tag="r2")
                    nc.scalar.activation(e[:], q[:], Act.Exp)
                    nc.vector.tensor_scalar_max(yb[:], yb[:], 0.0)
                    nc.vector.tensor_tensor(e[:], e[:], yb[:], AluOp.add)
                    nc.vector.tensor_scalar_add(
                        h2T[:, m, tl * 128:(tl + 1) * 128], e[:], -1.0)
            nc.sync.dma_start(alpha2o[:], alpha2sb[:])
            if DEBUG:
                nc.sync.dma_start(dbg_ag[:], ag_out[:])
                nc.sync.dma_start(dbg_h2[:], h2T[:])

        # ---------------- Phase F: transformer + pool ----------------
        with ExitStack() as sf_:
            sbF = sf_.enter_context(tc.tile_pool(name="sbF", bufs=1))
            sbFw = sf_.enter_context(tc.tile_pool(name="sbFw", bufs=1))
            psF = sf_.enter_context(tc.tile_pool(name="psF", bufs=4, space="PSUM"))
            psLN = sf_.enter_context(tc.tile_pool(name="psLN", bufs=2, space="PSUM"))

            Wv_s = sbF.tile([128, 2, 4, 512], bf16, name="Wv_sb")
            nc.sync.dma_start(Wv_s[:], Wv[:])
            Wo_s = sbF.tile([128, 2, 4, 512], bf16, name="Wo_sb")
            nc.sync.dma_start(Wo_s[:], Wo[:])
            W1f_s = sbF.tile([128, 2, 4, 2048], bf16, name="W1f_sb")
            nc.sync.dma_start(W1f_s[:], W1f[:])
            W2f_s = sbF.tile([128, 2, 16, 512], bf16, name="W2f_sb")
            nc.sync.dma_start(W2f_s[:], W2f[:])

            def loadcol(dram_t, shape):
                t = sbF.tile(list(shape), f32, name=dram_t.name + "_sb")
                nc.sync.dma_start(t[:], dram_t[:])
                return t

            bv_s = loadcol(bvcol, (128, 2, 4))
            bo_s = loadcol(bocol, (128, 2, 4))
            f1b_s = loadcol(f1bcol, (128, 2, 16))
            f2b_s = loadcol(f2bcol, (128, 2, 4))
            g1_s = loadcol(g1col, (128, 2, 4))
            h1c_s = loadcol(h1col, (128, 2, 4))
            g2_s = loadcol(g2col, (128, 2, 4))
            h2c_s = loadcol(h2col, (128, 2, 4))

            cur = h2T  # [128, 4, 512] bf16

            def dense(out_sb, W_s, i, nkc, nmc, rhs_tile, bias_s, act):
                """out[:, mc, :] = act(sum_kc W[:,i,kc,mc*128:...]^T @ rhs[:,kc,:] + b)"""
                for mc in range(nmc):
                    ps = psF.tile([128, 512], f32, name=f"mm{i}_{id(W_s)%97}_{mc}",
                                  tag="mmps")
                    for kc in range(nkc):
                        nc.tensor.matmul(
                            ps[:], W_s[:, i, kc, mc * 128:(mc + 1) * 128],
                            rhs_tile[:, kc, :], start=(kc == 0),
                            stop=(kc == nkc - 1))
                    nc.scalar.activation(out_sb[:, mc, :], ps[:], act,
                                         bias=bias_s[:, i, mc:mc + 1])

            def layernorm(r_sb, g_s, b_s, i, out_sb):
                sm = psLN.tile([1, 512], f32, name=f"sm{i}_{id(g_s)%97}", tag="sm")
                sq = psLN.tile([1, 512], f32, name=f"sq{i}_{id(g_s)%97}", tag="sm")
                sqv = sbFw.tile([128, 4, 512], bf16, name=f"sqv{i}", tag="sqv")
                for kc in range(4):
                    nc.vector.tensor_tensor(sqv[:, kc, :], r_sb[:, kc, :],
                                            r_sb[:, kc, :], AluOp.mult)
                for kc in range(4):
                    nc.tensor.matmul(sm[:], onesc_s[:], r_sb[:, kc, :],
                                     start=(kc == 0), stop=(kc == 3))
                    nc.tensor.matmul(sq[:], onesc_s[:], sqv[:, kc, :],
                                     start=(kc == 0), stop=(kc == 3))
                mu = sbFw.tile([1, 512], f32, name=f"mu{i}", tag="mu")
                nc.vector.tensor_scalar_mul(mu[:], sm[:], 1.0 / 512)
                ms = sbFw.tile([1, 512], f32, name=f"ms{i}", tag="ms")
                nc.vector.tensor_scalar_mul(ms[:], sq[:], 1.0 / 512)
                mu2 = sbFw.tile([1, 512], f32, name=f"mu2{i}", tag="mu2")
                nc.vector.tensor_tensor(mu2[:], mu[:], mu[:], AluOp.mult)
                nc.vector.tensor_tensor(ms[:], ms[:], mu2[:], AluOp.subtract)
                nc.vector.tensor_scalar_add(ms[:], ms[:], 1e-5)
                nc.scalar.activation(ms[:], ms[:], Act.Sqrt)
                inv = sbFw.tile([1, 512], f32, name=f"inv{i}", tag="inv")
                nc.vector.reciprocal(inv[:], ms[:])
                bcr = sbFw.tile([1, 1024], bf16, name=f"bcr{i}", tag="bcr")
                nc.vector.tensor_copy(bcr[:, 0:512], inv[:])
                nc.vector.tensor_tensor(mu2[:], mu[:], inv[:], AluOp.mult)
                nc.vector.tensor_scalar_mul(mu2[:], mu2[:], -1.0)
                nc.vector.tensor_copy(bcr[:, 512:1024], mu2[:])
                ib = psLN.tile([128, 512], f32, name=f"ib{i}", tag="ib")
                nc.tensor.matmul(ib[:], onesk_s[:], bcr[:, 0:512],
                                 start=True, stop=True)
                mb = psLN.tile([128, 512], f32, name=f"mb{i}", tag="ib")
                nc.tensor.matmul(mb[:], onesk_s[:], bcr[:, 512:1024],
                                 start=True, stop=True)
                for kc in range(4):
                    t1 = sbFw.tile([128, 512], f32, name=f"t1{i}_{kc}", tag="t1")
                    nc.vector.tensor_tensor(t1[:], r_sb[:, kc, :], ib[:],
                                            AluOp.mult)
                    nc.vector.tensor_tensor(t1[:], t1[:], mb[:], AluOp.add)
                    nc.vector.tensor_scalar(out_sb[:, kc, :], t1[:],
                                            g_s[:, i, kc:kc + 1],
                                            b_s[:, i, kc:kc + 1],
                                            AluOp.mult, AluOp.add)

            for i in range(2):
                vsb = sbFw.tile([128, 4, 512], bf16, name=f"v{i}", tag="vsb")
                dense(vsb, Wv_s, i, 4, 4, cur, bv_s, Act.Identity)
                asb = sbFw.tile([128, 4, 512], bf16, name=f"a{i}", tag="asb")
                dense(asb, Wo_s, i, 4, 4, vsb, bo_s, Act.Identity)
                rsb = sbFw.tile([128, 4, 512], bf16, name=f"rs{i}", tag="rsb")
                for kc in range(4):
                    nc.vector.tensor_tensor(rsb[:, kc, :], cur[:, kc, :],
                                            asb[:, kc, :], AluOp.add)
                ln1o = sbFw.tile([128, 4, 512], bf16, name=f"l1{i}", tag="ln1o")
                layernorm(rsb, g1_s, h1c_s, i, ln1o)
                f1sb = sbFw.tile([128, 16, 512], bf16, name=f"f1{i}", tag="f1sb")
                dense(f1sb, W1f_s, i, 4, 16, ln1o, f1b_s, Act.Relu)
                f2sb = sbFw.tile([128, 4, 512], bf16, name=f"f2{i}", tag="f2sb")
                dense(f2sb, W2f_s, i, 16, 4, f1sb, f2b_s, Act.Identity)
                r2sb = sbFw.tile([128, 4, 512], bf16, name=f"r2{i}", tag="r2sb")
                for kc in range(4):
                    nc.vector.tensor_tensor(r2sb[:, kc, :], ln1o[:, kc, :],
                                            f2sb[:, kc, :], AluOp.add)
                cur = sbFw.tile([128, 4, 512], bf16, name=f"cur{i}", tag="cur")
                layernorm(r2sb, g2_s, h2c_s, i, cur)

            poolsb = sbFw.tile([128, 4, 8], f32, name="poolsb", tag="pool")
            for kc in range(4):
                nc.vector.tensor_reduce(
                    poolsb[:, kc, :],
                    cur[:, kc, :].rearrange("p (g x) -> p g x", g=8),
                    mybir.AxisListType.X, AluOp.max)
            nc.sync.dma_start(pooled[:], poolsb[:])

    nc.compile()
    return nc


def kernel(**inputs):
    if "nc" not in _cache:
        _cache["nc"] = _build_nc()
    nc = _cache["nc"]
    in_maps, meta = _host_prep(inputs)
    res = bass_utils.run_bass_kernel_spmd(nc, in_maps, list(range(NDEV)))
    x_mean = np.zeros((B, D), np.float32)
    alpha2 = np.zeros((E + N, 1), np.float32)
    slot_eid = meta["slot_eid"]
    for d in range(NDEV):
        r = res.results[d]
        p = r["pooled"]          # [128, 4, 8]
        for g in range(8):
            x_mean[8 * d + g] = p[:, :, g].T.reshape(-1)
        a = r["alpha2o"]         # [128, C2C]
        base = 4 * d * NCH * 128
        for kk in range(C2C):
            gs = base + kk * 128 + np.arange(128)
            eids = slot_eid[gs]
            m = eids >= 0
            alpha2[eids[m], 0] = a[m, kk]
    return x_mean, alpha2


# revision 13
# speedup vs baseline: 1.0198x; 1.0198x over previous
"""Self-contained 8-core Trainium2 Bass kernel for nn_GATNet (GAT x2 + 2-layer
transformer (seq_len=1) + global max pool).

Strategy:
- GAT1 channel-sharded: each core computes xs1 = x1 @ W1[:, cols_d] for ALL
  nodes (640 of 5120 channels), writes rows to DRAM, dma_gathers per-edge rows
  (sorted by dst, tiled 128 dst-nodes / 6 chunks of 128 edge-slots), computes
  attention via one-hot scatter-matrix matmuls, aggregates on the PE.
- xs2 = h1 @ W2 partial per core -> ReduceScatter(+AllGather) in bf16.
- GAT2 + transformer + pooling node-sharded (512 nodes/core).
- All matmuls bf16 x bf16 -> f32 PSUM; softmax/LN arithmetic f32.
Host side does only: int index preprocessing (edge sort by dst), weight-only
constant folding, dtype casts/layout, and output reassembly.
"""
import sys
import types
import numpy as np
import ml_dtypes
from contextlib import ExitStack

import concourse.bass as bass
import concourse.tile as tile
from concourse import bacc, mybir
from concourse import bass_utils

try:  # optional NTFF profiling plumbing (no-op if unavailable)
    import antenv.axon_hooks  # noqa: F401
except ImportError:
    _h = types.ModuleType("antenv.axon_hooks")
    _h._hook = None
    _h.set_axon_ntff_profile_hook = lambda hk: setattr(_h, "_hook", hk)
    _h.get_axon_ntff_profile_hook = lambda: _h._hook
    sys.modules["antenv.axon_hooks"] = _h
    try:
        from trn_agent_boot.trn_boot import _ntff_profile_via_ctypes
        _h.set_axon_ntff_profile_hook(
            _ntff_profile_via_ctypes("/opt/axon/libaxon_pjrt.so"))
        bass_utils.upload_artifacts = lambda tmpdir: f"local://{tmpdir}"
    except Exception:
        pass

bf16 = mybir.dt.bfloat16
f32 = mybir.dt.float32
i16 = mybir.dt.int16

N, E, B = 4096, 16384, 64
H1, C, D = 10, 512, 512
NDEV = 8
CSH = 640            # GAT1 channels per device
CPH = CSH // H1      # 64 channels per head per device
NT = N // 128        # 32 node tiles
NCH = 6              # chunks (of 128 edge slots) per node tile, uniform
T1C = NT * NCH       # 192 total GAT1 chunks
LT = 4               # local node tiles per device
C2C = LT * NCH       # GAT2 chunks per device
AluOp = mybir.AluOpType
Act = mybir.ActivationFunctionType

_cache = {}


def _dev_cols(d):
    return np.concatenate(
        [h * C + np.arange(d * CPH, (d + 1) * CPH) for h in range(H1)])


def _wrap_idx(idx):
    """[n] int -> [128, n//16] int16 gather-index layout."""
    n = len(idx)
    w = np.zeros((128, n // 16), np.int16)
    base = idx.reshape(-1, 16).T.astype(np.int16)   # [16, n//16]
    for r in range(8):
        w[r * 16:(r + 1) * 16, :] = base
    return w


def _host_prep(inputs):
    ei = np.asarray(inputs["edge_index"])
    ea = np.asarray(inputs["edge_attr"]).astype(np.float32)
    src0, dst0 = ei[0].astype(np.int64), ei[1].astype(np.int64)

    src_all = np.concatenate([src0, np.arange(N)])
    dst_all = np.concatenate([dst0, np.arange(N)])
    isreal = np.concatenate([np.ones(E, np.float32), np.zeros(N, np.float32)])
    eid = np.arange(E + N)
    order = np.argsort(dst_all, kind="stable")
    src_s, dst_s, isreal_s, eid_s = (src_all[order], dst_all[order],
                                     isreal[order], eid[order])
    seg = np.searchsorted(dst_s, np.arange(0, N + 1, 128))

    NS = NT * NCH * 128
    slot_src = np.zeros(NS, np.int64)
    slot_dstloc = np.full(NS, 255.0, np.float32)
    slot_isreal = np.zeros(NS, np.float32)
    slot_isloop = np.zeros(NS, np.float32)
    slot_eid = np.full(NS, -1, np.int64)
    slot_ea = np.zeros((NS, 11), np.float32)
    ea_all = np.concatenate([ea, np.zeros((N, 11), np.float32)])
    for t in range(NT):
        lo, hi = seg[t], seg[t + 1]
        n = hi - lo
        assert n <= NCH * 128, f"tile {t} has {n} slots > {NCH*128}"
        s = t * NCH * 128
        slot_src[s:s + n] = src_s[lo:hi]
        slot_dstloc[s:s + n] = dst_s[lo:hi] - t * 128
        slot_isreal[s:s + n] = isreal_s[lo:hi]
        slot_isloop[s:s + n] = (isreal_s[lo:hi] == 0).astype(np.float32)
        slot_eid[s:s + n] = eid_s[lo:hi]
        slot_ea[s:s + n] = ea_all[eid_s[lo:hi]]

    # weight folds (f32 host math on weights only)
    W1 = np.asarray(inputs["W1"], np.float32)
    Ms = np.einsum("fhc,hc->fh", W1.reshape(93, H1, C),
                   np.asarray(inputs["as1"], np.float32))
    Md = np.einsum("fhc,hc->fh", W1.reshape(93, H1, C),
                   np.asarray(inputs["ad1"], np.float32))
    We1 = np.asarray(inputs["We1"], np.float32)
    Me1 = np.einsum("fhc,hc->fh", We1.reshape(11, H1, C),
                    np.asarray(inputs["ae1"], np.float32))
    W2 = np.asarray(inputs["W2"], np.float32)
    me2 = np.asarray(inputs["We2"], np.float32) @ np.asarray(
        inputs["ae2"], np.float32)[0]
    w2s = W2 @ np.asarray(inputs["as2"], np.float32)[0]
    w2d = W2 @ np.asarray(inputs["ad2"], np.float32)[0]

    # per-slot folded edge logit terms (fold of weights with edge attrs)
    se1 = slot_ea @ Me1                     # [NS, 10]
    se2 = slot_ea @ me2                     # [NS]

    def slotgrid(v, w=None):
        """[NS(,w)] -> [128, NT, NCH(,w)] partition layout."""
        if w is None:
            return np.ascontiguousarray(
                v.reshape(NT, NCH, 128).transpose(2, 0, 1))
        return np.ascontiguousarray(
            v.reshape(NT, NCH, 128, w).transpose(2, 0, 1, 3))

    b16 = ml_dtypes.bfloat16
    common = {
        "x1T": np.ascontiguousarray(
            np.asarray(inputs["x1"], np.float32).T).astype(b16),
        "easlot": np.concatenate(
            [slot_isreal[:, None], se1, se2[:, None],
             np.zeros((NS, 4), np.float32)], axis=1
        ).astype(b16).reshape(NT, NCH, 128, 16).transpose(2, 0, 1, 3).copy(),
        "se1f": slotgrid(se1, 10).astype(np.float32),
        "dstloc": slotgrid(slot_dstloc),
        "isloop": slotgrid(slot_isloop),
        "isloop10": np.repeat(slotgrid(slot_isloop)[:, :, :, None], 10, axis=3
                              ).astype(np.float32).copy(),
        "iota128": np.tile(np.arange(128, dtype=np.float32)[None, :], (128, 1)),
        "ident": np.eye(128, dtype=b16),
        "ones_k": np.ones((1, 128), b16),
        "onescol": np.ones((128, 1), b16),
        "idx1": _wrap_idx(slot_src),
        "b2col": np.asarray(inputs["b2"], np.float32).reshape(4, 128).T.copy(),
    }
    # transformer weights
    iw = np.asarray(inputs["in_w"], np.float32)
    Wv = iw[:, :, 2 * D:]                               # [2, 512, 512]
    common["Wv"] = np.ascontiguousarray(
        Wv.reshape(2, 4, 128, 512).transpose(2, 0, 1, 3)).astype(b16)
    Wo = np.asarray(inputs["out_w"], np.float32)
    common["Wo"] = np.ascontiguousarray(
        Wo.reshape(2, 4, 128, 512).transpose(2, 0, 1, 3)).astype(b16)
    W1f = np.asarray(inputs["f1w"], np.float32)
    common["W1f"] = np.ascontiguousarray(
        W1f.reshape(2, 4, 128, 2048).transpose(2, 0, 1, 3)).astype(b16)
    W2f = np.asarray(inputs["f2w"], np.float32)
    common["W2f"] = np.ascontiguousarray(
        W2f.reshape(2, 16, 128, 512).transpose(2, 0, 1, 3)).astype(b16)

    def col_layout(v, nc_):
        # [2, nc_*128] -> [128, 2, nc_]
        return np.ascontiguousarray(
            v.reshape(2, nc_, 128).transpose(2, 0, 1)).astype(np.float32)

    common["bvcol"] = col_layout(np.asarray(inputs["in_b"], np.float32)[:, 2 * D:], 4)
    common["bocol"] = col_layout(np.asarray(inputs["out_b"], np.float32), 4)
    common["f1bcol"] = col_layout(np.asarray(inputs["f1b"], np.float32), 16)
    common["f2bcol"] = col_layout(np.asarray(inputs["f2b"], np.float32), 4)
    common["g1col"] = col_layout(np.asarray(inputs["ln1g"], np.float32), 4)
    common["h1col"] = col_layout(np.asarray(inputs["ln1b"], np.float32), 4)
    common["g2col"] = col_layout(np.asarray(inputs["ln2g"], np.float32), 4)
    common["h2col"] = col_layout(np.asarray(inputs["ln2b"], np.float32), 4)

    in_maps = []
    b1 = np.asarray(inputs["b1"], np.float32)
    for d in range(NDEV):
        cols = _dev_cols(d)
        m = dict(common)
        m["W1aug"] = np.concatenate(
            [W1[:, cols], Ms, Md], axis=1).astype(b16)          # [93, 660]
        m["W2aug"] = np.ascontiguousarray(np.concatenate(
            [W2[cols], w2s[cols, None], w2d[cols, None],
             np.zeros((CSH, 2), np.float32)], axis=1
        ).reshape(5, 128, 516).transpose(1, 0, 2)).astype(b16)  # [128,5,516]
        m["b1col"] = np.ascontiguousarray(
            b1[cols].reshape(5, 128).T).astype(np.float32)      # [128, 5]
        # GAT2 per-core slot structure: local tiles = global tiles 4d..4d+3
        gsl = slice(4 * d * NCH * 128, (4 * d + 4) * NCH * 128)
        m["idx2"] = _wrap_idx(slot_src[gsl])
        m["dstloc2"] = slotgrid(slot_dstloc)[:, 4 * d:4 * d + 4, :].copy()
        m["isloop2"] = slotgrid(slot_isloop)[:, 4 * d:4 * d + 4, :].copy()
        m["ease2f"] = slotgrid(se2)[:, 4 * d:4 * d + 4, :].astype(np.float32).copy()
        in_maps.append(m)

    meta = dict(slot_eid=slot_eid)
    return in_maps, meta


DEBUG = False


def _build_nc():
    nc = bacc.Bacc("TRN2", target_bir_lowering=False, debug=True)

    def inp(name, shape, dt):
        return nc.dram_tensor(name, list(shape), dt, kind="ExternalInput")

    x1T = inp("x1T", (93, N), bf16)
    W1aug = inp("W1aug", (93, 660), bf16)
    easlot = inp("easlot", (128, NT, NCH, 16), bf16)
    se1f = inp("se1f", (128, NT, NCH, 10), f32)
    dstloc = inp("dstloc", (128, NT, NCH), f32)
    isloop = inp("isloop", (128, NT, NCH), f32)
    isloop10 = inp("isloop10", (128, NT, NCH, 10), f32)
    iota128 = inp("iota128", (128, 128), f32)
    ident = inp("ident", (128, 128), bf16)
    ones_k = inp("ones_k", (1, 128), bf16)
    onescol = inp("onescol", (128, 1), bf16)
    idx1 = inp("idx1", (128, T1C * 8), i16)
    idx2 = inp("idx2", (128, C2C * 8), i16)
    dstloc2 = inp("dstloc2", (128, LT, NCH), f32)
    isloop2 = inp("isloop2", (128, LT, NCH), f32)
    ease2f = inp("ease2f", (128, LT, NCH), f32)
    W2aug = inp("W2aug", (128, 5, 516), bf16)
    b1col = inp("b1col", (128, 5), f32)
    b2col = inp("b2col", (128, 4), f32)
    Wv = inp("Wv", (128, 2, 4, 512), bf16)
    Wo = inp("Wo", (128, 2, 4, 512), bf16)
    W1f = inp("W1f", (128, 2, 4, 2048), bf16)
    W2f = inp("W2f", (128, 2, 16, 512), bf16)
    bvcol = inp("bvcol", (128, 2, 4), f32)
    bocol = inp("bocol", (128, 2, 4), f32)
    f1bcol = inp("f1bcol", (128, 2, 16), f32)
    f2bcol = inp("f2bcol", (128, 2, 4), f32)
    g1col = inp("g1col", (128, 2, 4), f32)
    h1col = inp("h1col", (128, 2, 4), f32)
    g2col = inp("g2col", (128, 2, 4), f32)
    h2col = inp("h2col", (128, 2, 4), f32)

    pooled = nc.dram_tensor("pooled", [128, 4, 8], f32, kind="ExternalOutput")
    alpha2o = nc.dram_tensor("alpha2o", [128, C2C], f32, kind="ExternalOutput")
    if DEBUG:
        dbg_xs1 = nc.dram_tensor("dbg_xs1", [N, 768], bf16, kind="ExternalOutput")
        dbg_cc = nc.dram_tensor("dbg_cc", [N, 640], bf16, kind="ExternalOutput")
        dbg_ag = nc.dram_tensor("dbg_ag", [N, 640], bf16, kind="ExternalOutput")
        dbg_t0 = nc.dram_tensor("dbg_t0", [128, 64], f32, kind="ExternalOutput")
        dbg_g = nc.dram_tensor("dbg_g", [128, NCH, 768], bf16, kind="ExternalOutput")
        dbg_h1 = nc.dram_tensor("dbg_h1", [128, 5, N], bf16, kind="ExternalOutput")
        dbg_h2 = nc.dram_tensor("dbg_h2", [128, 4, 512], bf16, kind="ExternalOutput")
        dbg_ph = nc.dram_tensor("dbg_ph", [128, NCH, 10], f32, kind="ExternalOutput")

    xs1_dram = nc.dram_tensor("xs1_scratch", [N, 768], bf16)
    cc_in = nc.dram_tensor("cc_in", [N, 640], bf16)
    rs_out = nc.dram_tensor("rs_out", [N // NDEV, 640], bf16)
    ag_out = nc.dram_tensor("ag_out", [N, 640], bf16, addr_space="Shared")

    with tile.TileContext(nc) as tc, ExitStack() as top:
        cst = top.enter_context(tc.tile_pool(name="cst", bufs=1))
        mid_ctx = ExitStack()
        mid = mid_ctx.enter_context(tc.tile_pool(name="mid", bufs=1))

        def load(dram_t, shape, dt, pool=cst):
            t = pool.tile(list(shape), dt, name=dram_t.name + "_sb")
            nc.sync.dma_start(t[:], dram_t[:])
            return t

        x1T_s = load(x1T, (93, N), bf16, mid)
        W1aug_s = load(W1aug, (93, 660), bf16, mid)
        easlot_s = load(easlot, (128, NT, NCH, 16), bf16, mid)
        se1f_s = load(se1f, (128, NT, NCH, 10), f32, mid)
        dstloc_s = load(dstloc, (128, NT, NCH), f32, mid)
        isloop_s = load(isloop, (128, NT, NCH), f32, mid)
        isloop10_s = load(isloop10, (128, NT, NCH, 10), f32, mid)
        iota_s = load(iota128, (128, 128), f32)
        ident_s = load(ident, (128, 128), bf16)
        onesk_s = load(ones_k, (1, 128), bf16)
        onesc_s = load(onescol, (128, 1), bf16)
        idx1_s = load(idx1, (128, T1C * 8), i16, mid)
        W2aug_s = load(W2aug, (128, 5, 516), bf16, mid)
        b1col_s = load(b1col, (128, 5), f32, mid)

        # stats living through phases B..D only
        sdst_all = mid.tile([128, NT, 10], f32)     # s_dst1 per tile
        se2q_all = mid.tile([128, NT], f32)         # se2_loop/8 per tile
        h1T = mid.tile([128, 5, N], bf16)           # GAT1 output, T-layout

        # ---------------- Phase B: xs1 for all tiles ----------------
        with ExitStack() as sb_:
            psB = sb_.enter_context(tc.tile_pool(name="psB", bufs=2, space="PSUM"))
            sbB = sb_.enter_context(tc.tile_pool(name="sbB", bufs=3))
            for t in range(NT):
                ps = psB.tile([128, 660], f32, name=f"xs1ps{t}", tag="xs1ps")
                lhs = x1T_s[:, t * 128:(t + 1) * 128]
                nc.tensor.matmul(ps[:, 0:512], lhs, W1aug_s[:, 0:512],
                                 start=True, stop=True)
                nc.tensor.matmul(ps[:, 512:660], lhs, W1aug_s[:, 512:660],
                                 start=True, stop=True)
                xb = sbB.tile([128, 650], bf16, name=f"xs1b{t}", tag="xs1b")
                nc.scalar.copy(xb[:, 0:650], ps[:, 0:650])
                nc.vector.tensor_copy(sdst_all[:, t, :], ps[:, 650:660])
                nc.sync.dma_start(xs1_dram[t * 128:(t + 1) * 128, 0:650], xb[:])

        # ---------------- Phase A/C/D per tile ----------------
        with ExitStack() as sc_:
            sbS = sc_.enter_context(tc.tile_pool(name="sbS", bufs=2 * NCH + 2))
            sbG = sc_.enter_context(tc.tile_pool(name="sbG", bufs=3))
            sbW = sc_.enter_context(tc.tile_pool(name="sbW", bufs=3))
            sbU = sc_.enter_context(tc.tile_pool(name="sbU", bufs=2 * NCH + 2))
            psT = sc_.enter_context(tc.tile_pool(name="psT", bufs=2, space="PSUM"))
            psB2 = sc_.enter_context(tc.tile_pool(name="psB2", bufs=1, space="PSUM"))
            psS = sc_.enter_context(tc.tile_pool(name="psS", bufs=2, space="PSUM"))
            psA = sc_.enter_context(tc.tile_pool(name="psA", bufs=1, space="PSUM"))

            for t in range(NT):
                S_l, SkT_l = [], []
                esum = psS.tile([128, 12], f32, name=f"esum{t}", tag="stat")
                for j in range(NCH):
                    S_k = sbS.tile([128, 128], bf16, name=f"S{t}_{j}", tag="S")
                    nc.vector.tensor_scalar(
                        S_k[:], iota_s[:], dstloc_s[:, t, j:j + 1], None,
                        AluOp.is_equal)
                    tps = psT.tile([128, 128], bf16, name=f"tp{t}_{j}", tag="trbc")
                    nc.tensor.transpose(tps[:], S_k[:], ident_s[:])
                    SkT = sbS.tile([128, 128], bf16, name=f"ST{t}_{j}", tag="ST")
                    nc.scalar.copy(SkT[:], tps[:])
                    nc.tensor.matmul(esum[:], S_k[:], easlot_s[:, t, j, 0:12],
                                     start=(j == 0), stop=(j == NCH - 1))
                    S_l.append(S_k)
                    SkT_l.append(SkT)
                # loop stats
                recipc = sbW.tile([128, 1], f32, name=f"rc{t}", tag="rc")
                nc.vector.tensor_scalar_max(recipc[:], esum[:, 0:1], 1.0)
                nc.vector.reciprocal(recipc[:], recipc[:])
                sdaug = sbW.tile([128, 20], f32, name=f"sda{t}", tag="sda")
                nc.vector.tensor_copy(sdaug[:, 0:10], sdst_all[:, t, :])
                nc.vector.tensor_scalar_mul(sdaug[:, 10:20], esum[:, 1:11],
                                            recipc[:])
                nc.vector.tensor_scalar(se2q_all[:, t:t + 1], esum[:, 11:12],
                                        recipc[:], 0.125, AluOp.mult,
                                        AluOp.mult)
                sdaugb = sbW.tile([128, 20], bf16, name=f"sdab{t}", tag="sdab")
                nc.vector.tensor_copy(sdaugb[:], sdaug[:])

                # gather
                G = sbG.tile([128, NCH, 768], bf16, name=f"G{t}", tag="G")
                nc.gpsimd.dma_gather(
                    G[:], xs1_dram[:], idx1_s[:, t * NCH * 8:(t + 1) * NCH * 8],
                    NCH * 128, NCH * 128, 768)

                denom = psS.tile([128, 10], f32, name=f"den{t}", tag="stat")
                bca = psB2.tile([128, NCH, 20], f32, name=f"bca{t}", tag="bca")
                nc.vector.memset(bca[:], 0.0)
                for j in range(NCH):
                    nc.tensor.matmul(bca[:, j, :], SkT_l[j][:], sdaugb[:],
                                     start=False, stop=False,
                                     skip_group_check=True)
                u = sbU.tile([128, NCH, 10], f32, name=f"u{t}", tag="u")
                nc.vector.tensor_tensor(u[:], bca[:, :, 0:10],
                                        se1f_s[:, t, :, :], AluOp.add)
                u2 = sbU.tile([128, NCH, 10], f32, name=f"u2{t}", tag="u2")
                nc.vector.tensor_tensor(u2[:], bca[:, :, 10:20],
                                        isloop10_s[:, t, :, :], AluOp.mult)
                nc.vector.tensor_tensor(u[:], u[:], u2[:], AluOp.add)
                gs = sbU.tile([128, NCH, 10], f32, name=f"gs{t}", tag="gs")
                nc.scalar.copy(gs[:], G[:, :, 640:650])
                nc.vector.tensor_tensor(u[:], u[:], gs[:], AluOp.add)
                nc.vector.tensor_scalar_mul(u2[:], u[:], 0.2)
                nc.vector.tensor_tensor(u[:], u[:], u2[:], AluOp.max)
                ph = sbU.tile([128, NCH, 10], f32, name=f"ph{t}", tag="ph")
                nc.scalar.activation(ph[:], u[:], Act.Exp)
                phb = sbU.tile([128, NCH, 10], bf16, name=f"phb{t}", tag="phb")
                nc.vector.tensor_copy(phb[:], ph[:])
                for j in range(NCH):
                    nc.tensor.matmul(denom[:], S_l[j][:], phb[:, j, :],
                                     start=(j == 0), stop=(j == NCH - 1))
                if DEBUG and t == 0:
                    phat_l = [ph[:, _j, :] for _j in range(NCH)]
                recip = sbW.tile([128, 10], f32, name=f"rp{t}", tag="rp")
                nc.vector.reciprocal(recip[:], denom[:])
                recipb = sbW.tile([128, 10], bf16, name=f"rpb{t}", tag="rpb")
                nc.vector.tensor_copy(recipb[:], recip[:])
                if DEBUG and t == 0:
                    d0 = sbW.tile([128, 64], f32, name="dbg_t0sb", tag="d0")
                    nc.vector.tensor_copy(d0[:, 0:12], esum[:])
                    nc.vector.tensor_copy(d0[:, 12:32], sdaug[:])
                    nc.vector.tensor_copy(d0[:, 32:42], denom[:])
                    nc.vector.tensor_copy(d0[:, 42:52], recip[:])
                    nc.vector.tensor_copy(d0[:, 52:62], sdst_all[:, 0, :])
                    nc.vector.tensor_copy(d0[:, 62:63], se2q_all[:, 0:1])
                    nc.vector.memset(d0[:, 63:64], 0.0)
                    nc.sync.dma_start(dbg_t0[:], d0[:])
                    nc.sync.dma_start(dbg_g[:], G[:])
                    for _j in range(NCH):
                        nc.sync.dma_start(dbg_ph[:, _j, :], phat_l[_j][:])

                agg = psA.tile([128, 5, 128], f32, name=f"agg{t}", tag="agg")
                nc.vector.memset(agg[:], 0.0)
                rba = psB2.tile([128, NCH, 10], f32, name=f"rba{t}", tag="rba")
                nc.vector.memset(rba[:], 0.0)
                for j in range(NCH):
                    nc.tensor.matmul(rba[:, j, :], SkT_l[j][:], recipb[:],
                                     start=False, stop=False,
                                     skip_group_check=True)
                alb = sbU.tile([128, NCH, 10], bf16, name=f"alb{t}", tag="alb")
                nc.vector.tensor_tensor(alb[:], ph[:], rba[:], AluOp.mult)
                for j in range(NCH):
                    Gw = sbG.tile([128, 640], bf16, name=f"Gw{t}_{j}", tag="Gw")
                    nc.vector.tensor_tensor(
                        Gw[:].rearrange("p (h c) -> p h c", h=H1),
                        G[:, j, 0:640].rearrange("p (h c) -> p h c", h=H1),
                        alb[:, j, :, None].broadcast_to([128, H1, CPH]),
                        AluOp.mult)
                    for m in range(5):
                        nc.tensor.matmul(
                            agg[:, m, :], Gw[:, m * 128:(m + 1) * 128],
                            S_l[j][:], start=False, stop=False,
                            skip_group_check=True)
                # elu + bias -> h1T (batched over the 5 m-chunks)
                yb = sbW.tile([128, 5, 128], f32, name=f"yb{t}", tag="yb")
                nc.vector.tensor_tensor(
                    yb[:], agg[:],
                    b1col_s[:, :, None].broadcast_to([128, 5, 128]), AluOp.add)
                q = sbW.tile([128, 5, 128], f32, name=f"q{t}", tag="q")
                nc.vector.tensor_scalar_min(q[:], yb[:], 0.0)
                e = sbW.tile([128, 5, 128], f32, name=f"e{t}", tag="e")
                nc.scalar.activation(e[:], q[:], Act.Exp)
                nc.vector.tensor_scalar_max(yb[:], yb[:], 0.0)
                nc.vector.tensor_tensor(e[:], e[:], yb[:], AluOp.add)
                nc.vector.tensor_scalar_add(
                    h1T.rearrange("p m (nt x) -> p m nt x", x=128)[:, :, t, :],
                    e[:], -1.0)

                # Phase D: xs2 partial for this tile
                xs2p = psS.tile([128, 512], f32, name=f"xs2p{t}", tag="stat")
                xs2s = psS.tile([128, 4], f32, name=f"xs2s{t}", tag="stat")
                for kc in range(5):
                    lh = h1T[:, kc, t * 128:(t + 1) * 128]
                    nc.tensor.matmul(xs2p[:], lh, W2aug_s[:, kc, 0:512],
                                     start=(kc == 0), stop=(kc == 4))
                    nc.tensor.matmul(xs2s[:, 0:2], lh, W2aug_s[:, kc, 512:514],
                                     start=(kc == 0), stop=(kc == 4))
                ccs = sbG.tile([128, 640], bf16, name=f"cc{t}", tag="ccs")
                nc.scalar.copy(ccs[:, 0:512], xs2p[:])
                nc.scalar.copy(ccs[:, 512:514], xs2s[:, 0:2])
                nc.vector.tensor_copy(ccs[:, 514:515], se2q_all[:, t:t + 1])
                nc.vector.memset(ccs[:, 515:640], 0.0)
                nc.sync.dma_start(cc_in[t * 128:(t + 1) * 128, :], ccs[:])

        if DEBUG:
            nc.sync.dma_start(dbg_h1[:], h1T[:])
            nc.sync.dma_start(dbg_xs1[:], xs1_dram[:])
            nc.sync.dma_start(dbg_cc[:], cc_in[:])
        mid_ctx.close()

        # ---------------- collectives ----------------
        nc.gpsimd.collective_compute(
            "ReduceScatter", AluOp.add, replica_groups=[list(range(NDEV))],
            ins=[cc_in[:]], outs=[rs_out[:]])
        nc.gpsimd.collective_compute(
            "AllGather", AluOp.bypass, replica_groups=[list(range(NDEV))],
            ins=[rs_out[:]], outs=[ag_out[:]])

        # ---------------- Phase E: GAT2 ----------------
        with ExitStack() as se_:
            sbS2 = se_.enter_context(tc.tile_pool(name="sbS2", bufs=2 * NCH + 2))
            sbG2 = se_.enter_context(tc.tile_pool(name="sbG2", bufs=2))
            sbU2 = se_.enter_context(tc.tile_pool(name="sbU2", bufs=2 * NCH + 2))
            psT2 = se_.enter_context(tc.tile_pool(name="psT2", bufs=2, space="PSUM"))
            psS2 = se_.enter_context(tc.tile_pool(name="psS2", bufs=2, space="PSUM"))
            psA2 = se_.enter_context(tc.tile_pool(name="psA2", bufs=4, space="PSUM"))
            idx2_s = sbS2.tile([128, C2C * 8], i16, name="idx2_sb", tag="idx2")
            nc.sync.dma_start(idx2_s[:], idx2[:])
            dstloc2_s = sbS2.tile([128, LT, NCH], f32, name="dl2", tag="dl2")
            nc.sync.dma_start(dstloc2_s[:], dstloc2[:])
            isloop2_s = sbS2.tile([128, LT, NCH], f32, name="il2", tag="il2")
            nc.sync.dma_start(isloop2_s[:], isloop2[:])
            ease2_s = sbS2.tile([128, LT, NCH], f32, name="ea2", tag="ea2")
            nc.sync.dma_start(ease2_s[:], ease2f[:])
            b2c_s = sbS2.tile([128, 4], f32, name="b2c_s", tag="b2c")
            nc.sync.dma_start(b2c_s[:], b2col[:])
            sd2 = sbS2.tile([128, LT, 2], bf16, name="sd2", tag="sd2")
            for tl in range(LT):
                nc.sync.dma_start(sd2[:, tl, :],
                                  rs_out[tl * 128:(tl + 1) * 128, 513:515])
            h2T = cst.tile([128, 4, 512], bf16)
            alpha2sb = cst.tile([128, C2C], f32)
            for tl in range(LT):
                S_l, SkT_l = [], []
                for j in range(NCH):
                    S_k = sbS2.tile([128, 128], bf16, name=f"S2{tl}_{j}", tag="S2")
                    nc.vector.tensor_scalar(
                        S_k[:], iota_s[:], dstloc2_s[:, tl, j:j + 1], None,
                        AluOp.is_equal)
                    tps = psT2.tile([128, 128], bf16, name=f"tp2{tl}_{j}", tag="trbc2")
                    nc.tensor.transpose(tps[:], S_k[:], ident_s[:])
                    SkT = sbS2.tile([128, 128], bf16, name=f"ST2{tl}_{j}", tag="ST2")
                    nc.scalar.copy(SkT[:], tps[:])
                    S_l.append(S_k)
                    SkT_l.append(SkT)
                G2 = sbG2.tile([128, NCH, 640], bf16, name=f"G2{tl}", tag="G2")
                nc.gpsimd.dma_gather(
                    G2[:], ag_out[:], idx2_s[:, tl * NCH * 8:(tl + 1) * NCH * 8],
                    NCH * 128, NCH * 128, 640)
                den2 = psS2.tile([128, 1], f32, name=f"den2{tl}", tag="stat2")
                bc2a = psT2.tile([128, NCH, 2], f32, name=f"bc2a{tl}", tag="trbc2")
                nc.vector.memset(bc2a[:], 0.0)
                for j in range(NCH):
                    nc.tensor.matmul(bc2a[:, j, :], SkT_l[j][:], sd2[:, tl, :],
                                     start=False, stop=False,
                                     skip_group_check=True)
                u = sbU2.tile([128, NCH], f32, name=f"v{tl}", tag="v")
                nc.scalar.copy(u[:], G2[:, :, 512])
                nc.vector.tensor_tensor(u[:], u[:], bc2a[:, :, 0], AluOp.add)
                nc.vector.tensor_tensor(u[:], u[:], ease2_s[:, tl, :], AluOp.add)
                u2 = sbU2.tile([128, NCH], f32, name=f"v2{tl}", tag="v2")
                nc.vector.tensor_tensor(u2[:], bc2a[:, :, 1],
                                        isloop2_s[:, tl, :], AluOp.mult)
                nc.vector.tensor_tensor(u[:], u[:], u2[:], AluOp.add)
                nc.vector.tensor_scalar_mul(u2[:], u[:], 0.2)
                nc.vector.tensor_tensor(u[:], u[:], u2[:], AluOp.max)
                p2a = sbU2.tile([128, NCH], f32, name=f"p2a{tl}", tag="p2")
                nc.scalar.activation(p2a[:], u[:], Act.Exp)
                p2b = sbU2.tile([128, NCH], bf16, name=f"p2b{tl}", tag="p2b")
                nc.vector.tensor_copy(p2b[:], p2a[:])
                for j in range(NCH):
                    nc.tensor.matmul(den2[:], S_l[j][:], p2b[:, j:j + 1],
                                     start=(j == 0), stop=(j == NCH - 1))
                rc2 = sbU2.tile([128, 1], f32, name=f"rc2{tl}", tag="rc2")
                nc.vector.reciprocal(rc2[:], den2[:])
                rc2b = sbU2.tile([128, 1], bf16, name=f"rc2b{tl}", tag="rc2b")
                nc.vector.tensor_copy(rc2b[:], rc2[:])
                agg2 = [psA2.tile([128, 128], f32, name=f"agg2{tl}_{m}", tag="agg2")
                        for m in range(4)]
                rb2a = psT2.tile([128, NCH], f32, name=f"rb2a{tl}", tag="trbc2")
                nc.vector.memset(rb2a[:], 0.0)
                for j in range(NCH):
                    nc.tensor.matmul(rb2a[:, j:j + 1], SkT_l[j][:], rc2b[:],
                                     start=False, stop=False,
                                     skip_group_check=True)
                nc.vector.tensor_tensor(
                    alpha2sb[:, tl * NCH:(tl + 1) * NCH], p2a[:], rb2a[:],
                    AluOp.mult)
                for j in range(NCH):
                    kk = tl * NCH + j
                    Gw = sbG2.tile([128, 512], bf16, name=f"Gw2{tl}_{j}", tag="Gw2")
                    nc.vector.tensor_scalar_mul(Gw[:], G2[:, j, 0:512],
                                                alpha2sb[:, kk:kk + 1])
                    for m in range(4):
                        nc.tensor.matmul(
                            agg2[m][:], Gw[:, m * 128:(m + 1) * 128],
                            S_l[j][:], start=(j == 0), stop=(j == NCH - 1))
                for m in range(4):
                    yb = sbU2.tile([128, 128], f32, name=f"yb2{tl}_{m}", tag="q2")
                    nc.vector.tensor_scalar_add(yb[:], agg2[m][:],
                                                b2c_s[:, m:m + 1])
                    q = sbU2.tile([128, 128], f32, name=f"q2{tl}_{m}", tag="e2")
                    nc.vector.tensor_scalar_min(q[:], yb[:], 0.0)
                    e = sbU2.tile([128, 128], f32, name=f"e2b{tl}_{m}", tag="r2")
                    nc.scalar.activation(e[:], q[:], Act.Exp)
                    nc.vector.tensor_scalar_max(yb[:], yb[:], 0.0)
                    nc.vector.tensor_tensor(e[:], e[:], yb[:], AluOp.add)
                    nc.vector.tensor_scalar_add(
                        h2T[:, m, tl * 128:(tl + 1) * 128], e[:], -1.0)
            nc.sync.dma_start(alpha2o[:], alpha2sb[:])
            if DEBUG:
                nc.sync.dma_start(dbg_ag[:], ag_out[:])
                nc.sync.dma_start(dbg_h2[:], h2T[:])

        # ---------------- Phase F: transformer + pool ----------------
        with ExitStack() as sf_:
            sbF = sf_.enter_context(tc.tile_pool(name="sbF", bufs=1))
            sbFw = sf_.enter_context(tc.tile_pool(name="sbFw", bufs=1))
            psF = sf_.enter_context(tc.tile_pool(name="psF", bufs=4, space="PSUM"))
            psLN = sf_.enter_context(tc.tile_pool(name="psLN", bufs=2, space="PSUM"))

            Wv_s = sbF.tile([128, 2, 4, 512], bf16, name="Wv_sb")
            nc.sync.dma_start(Wv_s[:], Wv[:])
            Wo_s = sbF.tile([128, 2, 4, 512], bf16, name="Wo_sb")
            nc.sync.dma_start(Wo_s[:], Wo[:])
            W1f_s = sbF.tile([128, 2, 4, 2048], bf16, name="W1f_sb")
            nc.sync.dma_start(W1f_s[:], W1f[:])
            W2f_s = sbF.tile([128, 2, 16, 512], bf16, name="W2f_sb")
            nc.sync.dma_start(W2f_s[:], W2f[:])

            def loadcol(dram_t, shape):
                t = sbF.tile(list(shape), f32, name=dram_t.name + "_sb")
                nc.sync.dma_start(t[:], dram_t[:])
                return t

            bv_s = loadcol(bvcol, (128, 2, 4))
            bo_s = loadcol(bocol, (128, 2, 4))
            f1b_s = loadcol(f1bcol, (128, 2, 16))
            f2b_s = loadcol(f2bcol, (128, 2, 4))
            g1_s = loadcol(g1col, (128, 2, 4))
            h1c_s = loadcol(h1col, (128, 2, 4))
            g2_s = loadcol(g2col, (128, 2, 4))
            h2c_s = loadcol(h2col, (128, 2, 4))

            cur = h2T  # [128, 4, 512] bf16

            def dense(out_sb, W_s, i, nkc, nmc, rhs_tile, bias_s, act):
                """out[:, mc, :] = act(sum_kc W[:,i,kc,mc*128:...]^T @ rhs[:,kc,:] + b)"""
                for mc in range(nmc):
                    ps = psF.tile([128, 512], f32, name=f"mm{i}_{id(W_s)%97}_{mc}",
                                  tag="mmps")
                    for kc in range(nkc):
                        nc.tensor.matmul(
                            ps[:], W_s[:, i, kc, mc * 128:(mc + 1) * 128],
                            rhs_tile[:, kc, :], start=(kc == 0),
                            stop=(kc == nkc - 1))
                    nc.scalar.activation(out_sb[:, mc, :], ps[:], act,
                                         bias=bias_s[:, i, mc:mc + 1])

            def layernorm(r_sb, g_s, b_s, i, out_sb):
                sm = psLN.tile([1, 512], f32, name=f"sm{i}_{id(g_s)%97}", tag="sm")
                sq = psLN.tile([1, 512], f32, name=f"sq{i}_{id(g_s)%97}", tag="sm")
                sqv = sbFw.tile([128, 4, 512], bf16, name=f"sqv{i}", tag="sqv")
                for kc in range(4):
                    nc.vector.tensor_tensor(sqv[:, kc, :], r_sb[:, kc, :],
                                            r_sb[:, kc, :], AluOp.mult)
                for kc in range(4):
                    nc.tensor.matmul(sm[:], onesc_s[:], r_sb[:, kc, :],
                                     start=(kc == 0), stop=(kc == 3))
                    nc.tensor.matmul(sq[:], onesc_s[:], sqv[:, kc, :],
                                     start=(kc == 0), stop=(kc == 3))
                mu = sbFw.tile([1, 512], f32, name=f"mu{i}", tag="mu")
                nc.vector.tensor_scalar_mul(mu[:], sm[:], 1.0 / 512)
                ms = sbFw.tile([1, 512], f32, name=f"ms{i}", tag="ms")
                nc.vector.tensor_scalar_mul(ms[:], sq[:], 1.0 / 512)
                mu2 = sbFw.tile([1, 512], f32, name=f"mu2{i}", tag="mu2")
                nc.vector.tensor_tensor(mu2[:], mu[:], mu[:], AluOp.mult)
                nc.vector.tensor_tensor(ms[:], ms[:], mu2[:], AluOp.subtract)
                nc.vector.tensor_scalar_add(ms[:], ms[:], 1e-5)
                nc.scalar.activation(ms[:], ms[:], Act.Sqrt)
                inv = sbFw.tile([1, 512], f32, name=f"inv{i}", tag="inv")
                nc.vector.reciprocal(inv[:], ms[:])
                bcr = sbFw.tile([1, 1024], bf16, name=f"bcr{i}", tag="bcr")
                nc.vector.tensor_copy(bcr[:, 0:512], inv[:])
                nc.vector.tensor_tensor(mu2[:], mu[:], inv[:], AluOp.mult)
                nc.vector.tensor_scalar_mul(mu2[:], mu2[:], -1.0)
                nc.vector.tensor_copy(bcr[:, 512:1024], mu2[:])
                ib = psLN.tile([128, 512], f32, name=f"ib{i}", tag="ib")
                nc.tensor.matmul(ib[:], onesk_s[:], bcr[:, 0:512],
                                 start=True, stop=True)
                mb = psLN.tile([128, 512], f32, name=f"mb{i}", tag="ib")
                nc.tensor.matmul(mb[:], onesk_s[:], bcr[:, 512:1024],
                                 start=True, stop=True)
                for kc in range(4):
                    t1 = sbFw.tile([128, 512], f32, name=f"t1{i}_{kc}", tag="t1")
                    nc.vector.tensor_tensor(t1[:], r_sb[:, kc, :], ib[:],
                                            AluOp.mult)
                    nc.vector.tensor_tensor(t1[:], t1[:], mb[:], AluOp.add)
                    nc.vector.tensor_scalar(out_sb[:, kc, :], t1[:],
                                            g_s[:, i, kc:kc + 1],
                                            b_s[:, i, kc:kc + 1],
                                            AluOp.mult, AluOp.add)

            for i in range(2):
                vsb = sbFw.tile([128, 4, 512], bf16, name=f"v{i}", tag="vsb")
                dense(vsb, Wv_s, i, 4, 4, cur, bv_s, Act.Identity)
                asb = sbFw.tile([128, 4, 512], bf16, name=f"a{i}", tag="asb")
                dense(asb, Wo_s, i, 4, 4, vsb, bo_s, Act.Identity)
                rsb = sbFw.tile([128, 4, 512], bf16, name=f"rs{i}", tag="rsb")
                for kc in range(4):
                    nc.vector.tensor_tensor(rsb[:, kc, :], cur[:, kc, :],
                                            asb[:, kc, :], AluOp.add)
                ln1o = sbFw.tile([128, 4, 512], bf16, name=f"l1{i}", tag="ln1o")
                layernorm(rsb, g1_s, h1c_s, i, ln1o)
                f1sb = sbFw.tile([128, 16, 512], bf16, name=f"f1{i}", tag="f1sb")
                dense(f1sb, W1f_s, i, 4, 16, ln1o, f1b_s, Act.Relu)
                f2sb = sbFw.tile([128, 4, 512], bf16, name=f"f2{i}", tag="f2sb")
                dense(f2sb, W2f_s, i, 16, 4, f1sb, f2b_s, Act.Identity)
                r2sb = sbFw.tile([128, 4, 512], bf16, name=f"r2{i}", tag="r2sb")
                for kc in range(4):
                    nc.vector.tensor_tensor(r2sb[:, kc, :], ln1o[:, kc, :],
                                            f2sb[:, kc, :], AluOp.add)
                cur = sbFw.tile([128, 4, 512], bf16, name=f"cur{i}", tag="cur")
                layernorm(r2sb, g2_s, h2c_s, i, cur)

            poolsb = sbFw.tile([128, 4, 8], f32, name="poolsb", tag="pool")
            for kc in range(4):
                nc.vector.tensor_reduce(
                    poolsb[:, kc, :],
                    cur[:, kc, :].rearrange("p (g x) -> p g x", g=8),
                    mybir.AxisListType.X, AluOp.max)
            nc.sync.dma_start(pooled[:], poolsb[:])

    nc.compile()
    return nc


def kernel(**inputs):
    if "nc" not in _cache:
        _cache["nc"] = _build_nc()
    nc = _cache["nc"]
    in_maps, meta = _host_prep(inputs)
    res = bass_utils.run_bass_kernel_spmd(nc, in_maps, list(range(NDEV)))
    x_mean = np.zeros((B, D), np.float32)
    alpha2 = np.zeros((E + N, 1), np.float32)
    slot_eid = meta["slot_eid"]
    for d in range(NDEV):
        r = res.results[d]
        p = r["pooled"]          # [128, 4, 8]
        for g in range(8):
            x_mean[8 * d + g] = p[:, :, g].T.reshape(-1)
        a = r["alpha2o"]         # [128, C2C]
        base = 4 * d * NCH * 128
        for kk in range(C2C):
            gs = base + kk * 128 + np.arange(128)
            eids = slot_eid[gs]
            m = eids >= 0
            alpha2[eids[m], 0] = a[m, kk]
    return x_mean, alpha2
